# revision 6
# baseline (speedup 1.0000x reference)
"""ContextNet dynamic-conv kernel for 8 TRN2 NeuronCores — v2.

Math per sample b:
    gap[b]  = x[b].sum(T) / len[b]                  (C,)
    h[b]    = sigmoid(gap[b] @ w1.T + b1)           (2C,)
    w_dyn[b, co, ci, k] = h[b, 2*co + (ci>=C/2)] * W[co, ci, k]
    out[b]  = conv1d(x[b], w_dyn[b], pad=K//2)      (C, T)

Key restructure vs v1: the per-sample scale S[ci,co] = h[2co + (ci>=64)]
depends on ci only through its lo/hi half, so

    out[co,t] = h[2co]   * conv(x_lo, W_lo)[co,t]
              + h[2co+1] * conv(x_hi, W_hi)[co,t]

where W_lo/W_hi are BATCH-INDEPENDENT.  The conv runs as concurrent
64-deep row-tiled matmul pairs (PE tile_position (0,0)/(64,0), two PSUM
banks) with static bf16 weights, and h is applied at PSUM drain:
ACT does tmp = h_odd*psum_hi, DVE does out = h_even*psum_lo + tmp.
Nothing per-sample gates the PE anymore: sample 0's conv starts as soon
as its first x chunk is converted, ~10us in, instead of ~25us.  For
sample 0's first E0 tiles h isn't ready at drain time, so those drain
unscaled to SBUF (bf16) and are combined once h lands.

Sharding: pure data parallel over batch B=32 -> 4 samples per core x 8.
"""

import numpy as np
from contextlib import ExitStack

import concourse.bacc as bacc
import concourse.tile as tile
from concourse import mybir
from concourse.bass_utils import run_bass_kernel_spmd

B, C, T = 32, 128, 8192
K = 5
PAD = (K - 1) // 2
NCORES = 8
BL = B // NCORES          # samples per core
HC = C // 2
TT = 512                  # conv tile width (one PSUM bank of f32)
NT = T // TT              # 16 tiles per sample
OUT_GROUP = 4             # tiles per output DMA (512 KiB of bf16)
E0 = 12                   # sample-0 tiles drained unscaled (h not ready yet)

# input chunking: sample 0 fine-grained (PE start + h latency), rest coarse
S0_BOUNDS = [0, 512, 1536, 3072, 4608, 6144, 7680, 8192]
SB_BOUNDS = [0, 2048, 4096, 6144, 8192]
# per-chunk convert engine: A=ACT (accum fused), V=DVE (accum fused),
# P=Pool (no PSUM/no accum; row-sum via a later DVE bf16 reduce)
S0_CONV = ["A", "V", "P", "A", "V", "P", "A"]
SB_CONV = ["A", "V", "P", "P"]
XF_W = 2048               # staging tile width (max chunk size)

FP32 = mybir.dt.float32
BF16 = mybir.dt.bfloat16

AF = mybir.ActivationFunctionType
ALU = mybir.AluOpType
AXL = mybir.AxisListType


def build_nc():
    nc = bacc.Bacc("TRN2", target_bir_lowering=False, debug=False)

    x_d = nc.dram_tensor("x", [BL, C, T], FP32, kind="ExternalInput").ap()
    il_d = nc.dram_tensor("invlen", [1, BL], FP32, kind="ExternalInput").ap()
    w1t_d = nc.dram_tensor("w1t", [C, 2 * C], BF16, kind="ExternalInput").ap()
    b1_d = nc.dram_tensor("b1", [1, 2 * C], FP32, kind="ExternalInput").ap()
    wt_d = nc.dram_tensor("wt", [C, K * C], BF16, kind="ExternalInput").ap()
    ones_d = nc.dram_tensor("ones", [1, 64], BF16, kind="ExternalInput").ap()
    out_d = nc.dram_tensor("out", [BL, C, T], BF16, kind="ExternalOutput").ap()

    with ExitStack() as ctx:
        tc = ctx.enter_context(tile.TileContext(nc))

        const = ctx.enter_context(tc.tile_pool(name="const", bufs=1))
        xf = ctx.enter_context(tc.tile_pool(name="xf", bufs=8))
        xb = ctx.enter_context(tc.tile_pool(name="xb", bufs=2))
        outp = ctx.enter_context(tc.tile_pool(name="outp", bufs=3))
        tmpp = ctx.enter_context(tc.tile_pool(name="tmpp", bufs=3))
        earlyp = ctx.enter_context(tc.tile_pool(name="early", bufs=2 * E0))
        small = ctx.enter_context(tc.tile_pool(name="small", bufs=3))
        pconv = ctx.enter_context(tc.tile_pool(name="pconv", bufs=3, space="PSUM"))
        ps = ctx.enter_context(tc.tile_pool(name="ps", bufs=1, space="PSUM"))
        ph = ctx.enter_context(tc.tile_pool(name="ph", bufs=1, space="PSUM"))

        # constants ride the sync ring ahead of the x stream (small: ~230KB)
        wt_sb = const.tile([C, K * C], BF16)
        nc.sync.dma_start(wt_sb[:], wt_d[:])
        w1t_sb = const.tile([C, 2 * C], BF16)
        nc.sync.dma_start(w1t_sb[:], w1t_d[:])
        b1_sb = const.tile([1, 2 * C], FP32)
        nc.sync.dma_start(b1_sb[:], b1_d[:])
        il_sb = const.tile([1, BL], FP32)
        nc.sync.dma_start(il_sb[:], il_d[:])
        ones_sb = const.tile([1, 64], BF16)
        nc.sync.dma_start(ones_sb[:], ones_d[:])

        def emit_load(b):
            """DMA x[b] in chunks; convert to bf16 (+zero halo); row-sums."""
            first = b == 0
            bounds = S0_BOUNDS if first else SB_BOUNDS
            engs = S0_CONV if first else SB_CONV
            nch = len(bounds) - 1
            x_b = xb.tile([C, T + 2 * PAD], BF16)
            gap_parts = small.tile([C, 8], FP32, tag="gapp")
            nc.gpsimd.memset(x_b[:, 0:PAD], 0.0)
            nc.gpsimd.memset(x_b[:, T + PAD : T + 2 * PAD], 0.0)
            pool_chunks = []
            for c in range(nch):
                lo, hi = bounds[c], bounds[c + 1]
                w = hi - lo
                x_f = xf.tile([C, XF_W], FP32, tag="xfc")
                # sample 0: odd chunks ride the gpsimd ring so two rings
                # stream sample 0's data concurrently
                if first and c % 2 == 1:
                    nc.gpsimd.dma_start(x_f[:, 0:w], x_d[b, :, lo:hi])
                else:
                    nc.sync.dma_start(x_f[:, 0:w], x_d[b, :, lo:hi])
                dst = x_b[:, PAD + lo : PAD + hi]
                src = x_f[:, 0:w]
                if engs[c] == "A":
                    nc.scalar.activation(
                        dst, src, AF.Copy, accum_out=gap_parts[:, c : c + 1]
                    )
                elif engs[c] == "V":
                    nc.vector.tensor_scalar(
                        dst, src, 1.0, None,
                        op0=ALU.mult, op1=ALU.add,
                        accum_out=gap_parts[:, c : c + 1],
                    )
                else:
                    nc.gpsimd.tensor_copy(dst, src)
                    pool_chunks.append(c)
            for c in pool_chunks:
                lo, hi = bounds[c], bounds[c + 1]
                nc.vector.tensor_reduce(
                    gap_parts[:, c : c + 1],
                    x_b[:, PAD + lo : PAD + hi],
                    axis=AXL.X, op=ALU.add,
                )
            return x_b, gap_parts, nch

        def emit_h(b, gap_parts, nch):
            """h = sigmoid(gap @ w1.T * invlen + b1); h_even/h_odd as [C,1]."""
            gap_r = small.tile([C, 1], FP32, tag="gapr")
            nc.vector.tensor_reduce(
                gap_r[:], gap_parts[:, 0:nch], axis=AXL.X, op=ALU.add
            )
            gap_bf = small.tile([C, 1], BF16, tag="gapbf")
            nc.vector.tensor_copy(gap_bf[:], gap_r[:])
            h_ps = ph.tile([1, 2 * C], FP32)
            nc.tensor.matmul(
                h_ps[:], lhsT=gap_bf[:], rhs=w1t_sb[:], start=True, stop=True
            )
            h_pre = small.tile([1, 2 * C], FP32, tag="hpre")
            nc.vector.scalar_tensor_tensor(
                h_pre[:], h_ps[:], il_sb[0:1, b : b + 1], b1_sb[:],
                op0=ALU.mult, op1=ALU.add,
            )
            h_sb = small.tile([1, 2 * C], BF16, tag="h")
            nc.scalar.activation(h_sb[:], h_pre[:], AF.Sigmoid)
            # h_even/h_odd -> partition dim via contract-1 matmuls
            h3 = h_sb[:].rearrange("p (a two) -> p two a", two=2)  # (1, 2, 128)
            heho_ps = ps.tile([C, 2], FP32)
            nc.tensor.matmul(
                heho_ps[:, 0:1], lhsT=h3[:, 0, :], rhs=ones_sb[0:1, 0:1],
                start=True, stop=True, skip_group_check=True,
            )
            nc.tensor.matmul(
                heho_ps[:, 1:2], lhsT=h3[:, 1, :], rhs=ones_sb[0:1, 0:1],
                start=True, stop=True, skip_group_check=True,
            )
            heho_sb = small.tile([C, 2], FP32, tag="heho")
            nc.scalar.copy(heho_sb[:], heho_ps[:])
            return heho_sb

        def emit_conv_tile(x_b, t):
            """5 shifted 64-deep matmul pairs, concurrent via row tiling."""
            lo_ps = pconv.tile([C, TT], FP32, tag="lo")
            hi_ps = pconv.tile([C, TT], FP32, tag="hi")
            base = t * TT
            for k in range(K):
                nc.tensor.matmul(
                    lo_ps[:],
                    lhsT=wt_sb[0:HC, k * C : (k + 1) * C],
                    rhs=x_b[0:HC, base + k : base + k + TT],
                    start=(k == 0), stop=(k == K - 1),
                    skip_group_check=True,
                )
                nc.tensor.matmul(
                    hi_ps[:],
                    lhsT=wt_sb[HC:C, k * C : (k + 1) * C],
                    rhs=x_b[HC:C, base + k : base + k + TT],
                    start=(k == 0), stop=(k == K - 1),
                    skip_group_check=True,
                )
            return lo_ps, hi_ps

        def drain_steady(lo_ps, hi_ps, heho_sb, o_sb, off):
            """out = h_even*lo + h_odd*hi, ACT then DVE, psum -> bf16 sbuf."""
            tmp = tmpp.tile([C, TT], FP32, tag="t1")
            nc.scalar.mul(tmp[:], hi_ps[:], heho_sb[:, 1:2])
            nc.vector.scalar_tensor_tensor(
                o_sb[:, off : off + TT], lo_ps[:], heho_sb[:, 0:1], tmp[:],
                op0=ALU.mult, op1=ALU.add,
            )

        def drain_early(lo_ps, hi_ps):
            """h not ready yet: park psum in sbuf bf16."""
            el = earlyp.tile([C, TT], BF16, tag="el")
            eh = earlyp.tile([C, TT], BF16, tag="eh")
            nc.scalar.copy(eh[:], hi_ps[:])
            nc.vector.tensor_copy(el[:], lo_ps[:])
            return el, eh

        def combine_early(el, eh, heho_sb, o_sb, off, alt):
            u = tmpp.tile([C, TT], BF16, tag="u")
            nc.scalar.mul(u[:], eh[:], heho_sb[:, 1:2])
            nc.vector.scalar_tensor_tensor(
                o_sb[:, off : off + TT], el[:], heho_sb[:, 0:1], u[:],
                op0=ALU.mult, op1=ALU.add,
            )
            return

        def emit_out_dma(b, g, o_sb, split_tail):
            lo = g * OUT_GROUP * TT
            if split_tail:
                # drain the final group in 2-tile pieces so the kernel
                # tail isn't gated on one large DMA
                half = OUT_GROUP * TT // 2
                nc.gpsimd.dma_start(
                    out_d[b, :, lo : lo + half], o_sb[:, 0:half]
                )
                nc.gpsimd.dma_start(
                    out_d[b, :, lo + half : lo + 2 * half], o_sb[:, half:]
                )
            else:
                nc.gpsimd.dma_start(
                    out_d[b, :, lo : lo + OUT_GROUP * TT], o_sb[:]
                )

        # ---- sample 0: early tiles first, h chain, then the rest ----
        x_b0, gp0, nch0 = emit_load(0)
        early_bufs = []
        for t in range(E0):
            lo_ps, hi_ps = emit_conv_tile(x_b0, t)
            early_bufs.append(drain_early(lo_ps, hi_ps))
        heho0 = emit_h(0, gp0, nch0)
        x_b1, gp1, nch1 = emit_load(1)

        o_sbs0 = []
        for g in range(NT // OUT_GROUP):
            o_sb = outp.tile([C, OUT_GROUP * TT], BF16)
            o_sbs0.append(o_sb)
        # steady conv for the rest of sample 0 (PE priority: before combines)
        for t in range(E0, NT):
            lo_ps, hi_ps = emit_conv_tile(x_b0, t)
            drain_steady(
                lo_ps, hi_ps, heho0,
                o_sbs0[t // OUT_GROUP], (t % OUT_GROUP) * TT,
            )
        for t in range(E0):
            combine_early(
                early_bufs[t][0], early_bufs[t][1], heho0,
                o_sbs0[t // OUT_GROUP], (t % OUT_GROUP) * TT, alt=(t % 2 == 0),
            )
        for g in range(NT // OUT_GROUP):
            emit_out_dma(0, g, o_sbs0[g], split_tail=False)

        # ---- samples 1..BL-1: steady pipeline ----
        x_b_cur, gp_cur, nch_cur = x_b1, gp1, nch1
        for b in range(1, BL):
            heho = emit_h(b, gp_cur, nch_cur)
            if b + 1 < BL:
                x_b_n, gp_n, nch_n = emit_load(b + 1)
            last = b == BL - 1
            for g in range(NT // OUT_GROUP):
                o_sb = outp.tile([C, OUT_GROUP * TT], BF16)
                for j in range(OUT_GROUP):
                    t = g * OUT_GROUP + j
                    lo_ps, hi_ps = emit_conv_tile(x_b_cur, t)
                    drain_steady(lo_ps, hi_ps, heho, o_sb, j * TT)
                emit_out_dma(
                    b, g, o_sb,
                    split_tail=(last and g == NT // OUT_GROUP - 1),
                )
            if b + 1 < BL:
                x_b_cur, gp_cur, nch_cur = x_b_n, gp_n, nch_n

    nc.compile()
    return nc


_NC_CACHE = None


def _get_nc():
    global _NC_CACHE
    if _NC_CACHE is None:
        _NC_CACHE = build_nc()
    return _NC_CACHE


def make_in_maps(x, input_lengths, w1, b1, w2):
    import ml_dtypes

    x = np.ascontiguousarray(np.asarray(x, dtype=np.float32))
    lens = np.asarray(input_lengths).astype(np.float64)
    invlen = (1.0 / lens).astype(np.float32)
    w1t = np.ascontiguousarray(
        np.asarray(w1, dtype=np.float32).T.astype(ml_dtypes.bfloat16)
    )  # (C, 2C) bf16
    b1r = np.asarray(b1, dtype=np.float32).reshape(1, 2 * C)
    # wt[ci, k*C + co] = W[co, ci, k],  W = w2.reshape(C, C, K)
    wt = np.ascontiguousarray(
        np.asarray(w2, dtype=np.float32)
        .reshape(C, C, K)
        .transpose(1, 2, 0)
        .reshape(C, K * C)
        .astype(ml_dtypes.bfloat16)
    )
    ones = np.ones((1, 64), dtype=ml_dtypes.bfloat16)

    in_maps = []
    for i in range(NCORES):
        sl = slice(i * BL, (i + 1) * BL)
        in_maps.append(
            {
                "x": np.ascontiguousarray(x[sl]),
                "invlen": np.ascontiguousarray(invlen[sl].reshape(1, BL)),
                "w1t": w1t,
                "b1": b1r,
                "wt": wt,
                "ones": ones,
            }
        )
    return in_maps


def kernel(x, input_lengths, w1, b1, w2, _trace=False):
    nc = _get_nc()
    in_maps = make_in_maps(x, input_lengths, w1, b1, w2)
    res = run_bass_kernel_spmd(nc, in_maps, core_ids=list(range(NCORES)), trace=_trace)
    out = np.concatenate(
        [res.results[i]["out"].astype(np.float32) for i in range(NCORES)], axis=0
    )
    if _trace:
        kernel.last_exec_time_ns = res.exec_time_ns
        kernel.last_results = res
    return out


# revision 10
# speedup vs baseline: 1.2225x; 1.2225x over previous
"""ContextNet dynamic-conv kernel for 8 TRN2 NeuronCores — v3 (hybrid).

Math per sample b:
    gap[b]  = x[b].sum(T) / len[b]                  (C,)
    h[b]    = sigmoid(gap[b] @ w1.T + b1)           (2C,)
    w_dyn[b, co, ci, k] = h[b, 2*co + (ci>=C/2)] * W[co, ci, k]
    out[b]  = conv1d(x[b], w_dyn[b], pad=K//2)      (C, T)

Two conv modes:
  A (steady, 52 of 64 tiles): per-sample scaled weights wsc = W * S_b
    (S_b[ci,co] = h[b, 2co+(ci>=64)]), 5 shifted 128-deep bf16 matmuls
    per 512-col tile, one ACT psum->sbuf(bf16) drain per 2 tiles.  This
    is the cheapest drain path but needs h[b] before any matmul.
  B (sample 0's first E0 tiles): h[0] isn't ready until ~18us (needs the
    full-T sum), but S depends on ci only through its lo/hi half, so
    out = h[2co]*conv(x_lo,W_lo) + h[2co+1]*conv(x_hi,W_hi) with STATIC
    weights.  Those tiles run as row-tiled 64-deep matmul pairs
    (tile_position (0,0)/(64,0)) as soon as each x chunk is converted
    (~10us), drain unscaled to sbuf bf16, and are combined with h[0]
    later (spread over samples 1..3).

Sharding: pure data parallel over batch B=32 -> 4 samples per core x 8.
"""

import numpy as np
from contextlib import ExitStack

import concourse.bacc as bacc
import concourse.tile as tile
from concourse import mybir
from concourse.bass_utils import run_bass_kernel_spmd

B, C, T = 32, 128, 8192
K = 5
PAD = (K - 1) // 2
NCORES = 8
BL = B // NCORES          # samples per core
HC = C // 2
TT = 512                  # conv tile width (one PSUM bank of f32)
NT = T // TT              # 16 tiles per sample
OUT_GROUP = 4             # tiles per output DMA (512 KiB of bf16)
E0 = 12                   # sample-0 tiles run in static-weight B mode

# sample-0 chunks: fine-grained, boundaries at tile*512+4 so each conv
# tile's 4-col halo stays within one chunk; alternate sync/gpsimd rings.
S0_BOUNDS = [0, 516, 1540, 3076, 4612, 6148, 7684, 8192]
S0_CONV = ["A", "V", "A", "V", "A", "V", "A"]
# steady samples: coarse chunks, ACT+DVE converts (fused gap accum)
SB_BOUNDS = [0, 2048, 4096, 6144, 8192]
SB_CONV = ["A", "V", "V", "V"]
XF_W = 2048               # staging tile width (max chunk size)

FP32 = mybir.dt.float32
BF16 = mybir.dt.bfloat16

AF = mybir.ActivationFunctionType
ALU = mybir.AluOpType
AXL = mybir.AxisListType


def build_nc():
    nc = bacc.Bacc("TRN2", target_bir_lowering=False, debug=False)

    x_d = nc.dram_tensor("x", [BL, C, T], FP32, kind="ExternalInput").ap()
    il_d = nc.dram_tensor("invlen", [1, BL], FP32, kind="ExternalInput").ap()
    w1t_d = nc.dram_tensor("w1t", [C, 2 * C], BF16, kind="ExternalInput").ap()
    b1_d = nc.dram_tensor("b1", [1, 2 * C], FP32, kind="ExternalInput").ap()
    wt_d = nc.dram_tensor("wt", [C, K * C], BF16, kind="ExternalInput").ap()
    ones_d = nc.dram_tensor("ones", [1, 64], BF16, kind="ExternalInput").ap()
    out_d = nc.dram_tensor("out", [BL, C, T], BF16, kind="ExternalOutput").ap()

    with ExitStack() as ctx:
        tc = ctx.enter_context(tile.TileContext(nc))

        const = ctx.enter_context(tc.tile_pool(name="const", bufs=1))
        xf = ctx.enter_context(tc.tile_pool(name="xf", bufs=8))
        xb = ctx.enter_context(tc.tile_pool(name="xb", bufs=2))
        outp = ctx.enter_context(tc.tile_pool(name="outp", bufs=6))
        tmpp = ctx.enter_context(tc.tile_pool(name="tmpp", bufs=3))
        earlyp = ctx.enter_context(tc.tile_pool(name="early", bufs=E0))
        wscp = ctx.enter_context(tc.tile_pool(name="wscp", bufs=2))
        small = ctx.enter_context(tc.tile_pool(name="small", bufs=3))
        pc = ctx.enter_context(tc.tile_pool(name="pc", bufs=3, space="PSUM"))
        ps = ctx.enter_context(tc.tile_pool(name="ps", bufs=1, space="PSUM"))
        ph = ctx.enter_context(tc.tile_pool(name="ph", bufs=1, space="PSUM"))

        # constants ride the sync ring ahead of the x stream (small: ~230KB)
        wt_sb = const.tile([C, K * C], BF16)
        nc.sync.dma_start(wt_sb[:], wt_d[:])
        w1t_sb = const.tile([C, 2 * C], BF16)
        nc.sync.dma_start(w1t_sb[:], w1t_d[:])
        b1_sb = const.tile([1, 2 * C], FP32)
        nc.sync.dma_start(b1_sb[:], b1_d[:])
        il_sb = const.tile([1, BL], FP32)
        nc.sync.dma_start(il_sb[:], il_d[:])
        ones_sb = const.tile([1, 64], BF16)
        nc.sync.dma_start(ones_sb[:], ones_d[:])

        def emit_load(b):
            """DMA x[b] in chunks; convert to bf16 (+zero halo); row-sums."""
            first = b == 0
            bounds = S0_BOUNDS if first else SB_BOUNDS
            engs = S0_CONV if first else SB_CONV
            nch = len(bounds) - 1
            x_b = xb.tile([C, T + 2 * PAD], BF16)
            gap_parts = small.tile([C, 8], FP32, tag="gapp")
            nc.gpsimd.memset(x_b[:, 0:PAD], 0.0)
            nc.gpsimd.memset(x_b[:, T + PAD : T + 2 * PAD], 0.0)
            for c in range(nch):
                lo, hi = bounds[c], bounds[c + 1]
                w = hi - lo
                x_f = xf.tile([C, XF_W], FP32, tag="xfc")
                # sample 0: odd chunks ride the gpsimd ring so two rings
                # stream sample 0's data concurrently
                if first and c % 2 == 1:
                    nc.gpsimd.dma_start(x_f[:, 0:w], x_d[b, :, lo:hi])
                else:
                    nc.sync.dma_start(x_f[:, 0:w], x_d[b, :, lo:hi])
                dst = x_b[:, PAD + lo : PAD + hi]
                src = x_f[:, 0:w]
                if engs[c] == "A":
                    nc.scalar.activation(
                        dst, src, AF.Copy, accum_out=gap_parts[:, c : c + 1]
                    )
                else:
                    nc.vector.tensor_scalar(
                        dst, src, 1.0, None,
                        op0=ALU.mult, op1=ALU.add,
                        accum_out=gap_parts[:, c : c + 1],
                    )
            return x_b, gap_parts, nch

        def emit_h(b, gap_parts, nch):
            """h_sb = sigmoid(gap @ w1.T * invlen + b1) as [1,2C] bf16."""
            gap_r = small.tile([C, 1], FP32, tag="gapr")
            nc.vector.tensor_reduce(
                gap_r[:], gap_parts[:, 0:nch], axis=AXL.X, op=ALU.add
            )
            gap_bf = small.tile([C, 1], BF16, tag="gapbf")
            nc.vector.tensor_copy(gap_bf[:], gap_r[:])
            hb = ph.tile([C, 2 * C + 2], FP32)
            h_ps = hb[0:1, 0 : 2 * C]
            nc.tensor.matmul(
                h_ps, lhsT=gap_bf[:], rhs=w1t_sb[:],
                start=True, stop=True, skip_group_check=True,
            )
            h_pre = small.tile([1, 2 * C], FP32, tag="hpre")
            nc.vector.scalar_tensor_tensor(
                h_pre[:], h_ps, il_sb[0:1, b : b + 1], b1_sb[:],
                op0=ALU.mult, op1=ALU.add,
            )
            h_sb = small.tile([1, 2 * C], BF16, tag="h")
            nc.scalar.activation(h_sb[:], h_pre[:], AF.Sigmoid)
            return h_sb, hb

        def emit_heho(h_sb, hb):
            """h_even/h_odd as per-partition [C,1] f32 columns (sample 0)."""
            h3 = h_sb[:].rearrange("p (a two) -> p two a", two=2)  # (1, 2, 128)
            heho_ps = hb[:, 2 * C : 2 * C + 2]
            nc.tensor.matmul(
                heho_ps[:, 0:1], lhsT=h3[:, 0, :], rhs=ones_sb[0:1, 0:1],
                start=True, stop=True, skip_group_check=True,
            )
            nc.tensor.matmul(
                heho_ps[:, 1:2], lhsT=h3[:, 1, :], rhs=ones_sb[0:1, 0:1],
                start=True, stop=True, skip_group_check=True,
            )
            heho_sb = small.tile([C, 2], FP32, tag="heho")
            nc.scalar.copy(heho_sb[:], heho_ps[:])
            return heho_sb

        def emit_wsc(h_sb):
            """S_b[ci,co] = h[2co + (ci>=64)] broadcast; wsc = W * S_b."""
            h3 = h_sb[:].rearrange("p (a two) -> p two a", two=2)  # (1, 2, 128)
            s_ps = ps.tile([C, C], FP32)
            nc.tensor.matmul(
                s_ps[0:64, :], lhsT=ones_sb[:], rhs=h3[:, 0, :],
                start=True, stop=True, skip_group_check=True,
            )
            nc.tensor.matmul(
                s_ps[64:128, :], lhsT=ones_sb[:], rhs=h3[:, 1, :],
                start=True, stop=True, skip_group_check=True,
            )
            wsc = wscp.tile([C, K * C], BF16)
            for k in range(K):
                nc.vector.tensor_mul(
                    wsc[:, k * C : (k + 1) * C],
                    wt_sb[:, k * C : (k + 1) * C],
                    s_ps[:],
                )
            return wsc

        def emit_conv_tile_b(x_b, t, buf):
            """B mode: 5 shifted 64-deep pairs, concurrent via row tiling."""
            base = t * TT
            for k in range(K):
                nc.tensor.matmul(
                    buf[:, 0:TT],
                    lhsT=wt_sb[0:HC, k * C : (k + 1) * C],
                    rhs=x_b[0:HC, base + k : base + k + TT],
                    start=(k == 0), stop=(k == K - 1),
                    skip_group_check=True,
                )
                nc.tensor.matmul(
                    buf[:, TT : 2 * TT],
                    lhsT=wt_sb[HC:C, k * C : (k + 1) * C],
                    rhs=x_b[HC:C, base + k : base + k + TT],
                    start=(k == 0), stop=(k == K - 1),
                    skip_group_check=True,
                )

        def emit_conv_pair_a(x_b, wsc, t0, buf):
            """A mode: tiles t0,t0+1 as 128-deep matmuls into one psum buf."""
            for half in range(2):
                base = (t0 + half) * TT
                for k in range(K):
                    nc.tensor.matmul(
                        buf[:, half * TT : (half + 1) * TT],
                        lhsT=wsc[:, k * C : (k + 1) * C],
                        rhs=x_b[:, base + k : base + k + TT],
                        start=(k == 0), stop=(k == K - 1),
                        skip_group_check=True,
                    )

        def drain_early(buf):
            """B mode, h not ready yet: park both psum halves in sbuf bf16."""
            eb = earlyp.tile([C, 2 * TT], BF16, tag="e")
            nc.scalar.copy(eb[:, TT : 2 * TT], buf[:, TT : 2 * TT])
            nc.vector.tensor_copy(eb[:, 0:TT], buf[:, 0:TT])
            return eb

        def combine_early(eb, heho_sb, o_sb, off):
            """out = h_even*lo + h_odd*hi from the parked bf16 halves."""
            u = tmpp.tile([C, TT], BF16, tag="u")
            nc.scalar.mul(u[:], eb[:, TT : 2 * TT], heho_sb[:, 1:2])
            nc.vector.scalar_tensor_tensor(
                o_sb[:, off : off + TT], eb[:, 0:TT], heho_sb[:, 0:1], u[:],
                op0=ALU.mult, op1=ALU.add,
            )

        def emit_out_dma(b, g, o_sb, split_tail):
            lo = g * OUT_GROUP * TT
            if split_tail:
                # drain the final group in 2-tile pieces so the kernel
                # tail isn't gated on one large DMA
                half = OUT_GROUP * TT // 2
                nc.gpsimd.dma_start(out_d[b, :, lo : lo + half], o_sb[:, 0:half])
                nc.gpsimd.dma_start(
                    out_d[b, :, lo + half : lo + 2 * half], o_sb[:, half:]
                )
            else:
                nc.gpsimd.dma_start(
                    out_d[b, :, lo : lo + OUT_GROUP * TT], o_sb[:]
                )

        # ---- sample 0 ----
        x_b0, gp0, nch0 = emit_load(0)
        early_bufs = []
        for t in range(E0):
            buf = pc.tile([C, 2 * TT], FP32)
            emit_conv_tile_b(x_b0, t, buf)
            early_bufs.append(drain_early(buf))
        h0, hb0 = emit_h(0, gp0, nch0)
        heho0 = emit_heho(h0, hb0)
        wsc0 = emit_wsc(h0)
        x_b1, gp1, nch1 = emit_load(1)

        o_sbs0 = [
            outp.tile([C, OUT_GROUP * TT], BF16, name="osb0", tag="osb")
            for _ in range(NT // OUT_GROUP)
        ]
        # steady A-mode conv for the rest of sample 0
        for t0 in range(E0, NT, 2):
            buf = pc.tile([C, 2 * TT], FP32)
            emit_conv_pair_a(x_b0, wsc0, t0, buf)
            g = t0 // OUT_GROUP
            nc.scalar.copy(
                o_sbs0[g][:, (t0 % OUT_GROUP) * TT : (t0 % OUT_GROUP + 2) * TT],
                buf[:],
            )
        emit_out_dma(0, 3, o_sbs0[3], split_tail=False)

        # ---- samples 1..BL-1 (A mode); sample-0 combines spread across ----
        x_b_cur, gp_cur, nch_cur = x_b1, gp1, nch1
        for b in range(1, BL):
            h_b, _ = emit_h(b, gp_cur, nch_cur)
            wsc = emit_wsc(h_b)
            if b + 1 < BL:
                x_b_n, gp_n, nch_n = emit_load(b + 1)
            # 4 of sample 0's early combines per later sample
            for t in range((b - 1) * 4, min(b * 4, E0)):
                combine_early(
                    early_bufs[t], heho0,
                    o_sbs0[t // OUT_GROUP], (t % OUT_GROUP) * TT,
                )
            if b * 4 <= E0:
                emit_out_dma(0, b - 1, o_sbs0[b - 1], split_tail=False)
            last = b == BL - 1
            for g in range(NT // OUT_GROUP):
                o_sb = outp.tile([C, OUT_GROUP * TT], BF16, name="osb", tag="osb")
                for j2 in range(0, OUT_GROUP, 2):
                    buf = pc.tile([C, 2 * TT], FP32)
                    emit_conv_pair_a(x_b_cur, wsc, g * OUT_GROUP + j2, buf)
                    nc.scalar.copy(o_sb[:, j2 * TT : (j2 + 2) * TT], buf[:])
                emit_out_dma(
                    b, g, o_sb, split_tail=(last and g == NT // OUT_GROUP - 1)
                )
            if b + 1 < BL:
                x_b_cur, gp_cur, nch_cur = x_b_n, gp_n, nch_n

    nc.compile()
    return nc


_NC_CACHE = None


def _get_nc():
    global _NC_CACHE
    if _NC_CACHE is None:
        _NC_CACHE = build_nc()
    return _NC_CACHE


def make_in_maps(x, input_lengths, w1, b1, w2):
    import ml_dtypes

    x = np.ascontiguousarray(np.asarray(x, dtype=np.float32))
    lens = np.asarray(input_lengths).astype(np.float64)
    invlen = (1.0 / lens).astype(np.float32)
    w1t = np.ascontiguousarray(
        np.asarray(w1, dtype=np.float32).T.astype(ml_dtypes.bfloat16)
    )  # (C, 2C) bf16
    b1r = np.asarray(b1, dtype=np.float32).reshape(1, 2 * C)
    # wt[ci, k*C + co] = W[co, ci, k],  W = w2.reshape(C, C, K)
    wt = np.ascontiguousarray(
        np.asarray(w2, dtype=np.float32)
        .reshape(C, C, K)
        .transpose(1, 2, 0)
        .reshape(C, K * C)
        .astype(ml_dtypes.bfloat16)
    )
    ones = np.ones((1, 64), dtype=ml_dtypes.bfloat16)

    in_maps = []
    for i in range(NCORES):
        sl = slice(i * BL, (i + 1) * BL)
        in_maps.append(
            {
                "x": np.ascontiguousarray(x[sl]),
                "invlen": np.ascontiguousarray(invlen[sl].reshape(1, BL)),
                "w1t": w1t,
                "b1": b1r,
                "wt": wt,
                "ones": ones,
            }
        )
    return in_maps


def kernel(x, input_lengths, w1, b1, w2, _trace=False):
    nc = _get_nc()
    in_maps = make_in_maps(x, input_lengths, w1, b1, w2)
    res = run_bass_kernel_spmd(nc, in_maps, core_ids=list(range(NCORES)), trace=_trace)
    out = np.concatenate(
        [res.results[i]["out"].astype(np.float32) for i in range(NCORES)], axis=0
    )
    if _trace:
        kernel.last_exec_time_ns = res.exec_time_ns
        kernel.last_results = res
    return out


# revision 12
# speedup vs baseline: 1.3684x; 1.1194x over previous
"""ContextNet dynamic-conv kernel for 8 TRN2 NeuronCores — v3 (hybrid).

Math per sample b:
    gap[b]  = x[b].sum(T) / len[b]                  (C,)
    h[b]    = sigmoid(gap[b] @ w1.T + b1)           (2C,)
    w_dyn[b, co, ci, k] = h[b, 2*co + (ci>=C/2)] * W[co, ci, k]
    out[b]  = conv1d(x[b], w_dyn[b], pad=K//2)      (C, T)

Two conv modes:
  A (steady, 52 of 64 tiles): per-sample scaled weights wsc = W * S_b
    (S_b[ci,co] = h[b, 2co+(ci>=64)]), 5 shifted 128-deep bf16 matmuls
    per 512-col tile, one ACT psum->sbuf(bf16) drain per 2 tiles.  This
    is the cheapest drain path but needs h[b] before any matmul.
  B (sample 0's first E0 tiles): h[0] isn't ready until ~18us (needs the
    full-T sum), but S depends on ci only through its lo/hi half, so
    out = h[2co]*conv(x_lo,W_lo) + h[2co+1]*conv(x_hi,W_hi) with STATIC
    weights.  Those tiles run as row-tiled 64-deep matmul pairs
    (tile_position (0,0)/(64,0)) as soon as each x chunk is converted
    (~10us), drain unscaled to sbuf bf16, and are combined with h[0]
    later (spread over samples 1..3).

Sharding: pure data parallel over batch B=32 -> 4 samples per core x 8.
"""

import numpy as np
from contextlib import ExitStack

import concourse.bacc as bacc
import concourse.tile as tile
from concourse import mybir
from concourse.bass_utils import run_bass_kernel_spmd

B, C, T = 32, 128, 8192
K = 5
PAD = (K - 1) // 2
NCORES = 8
BL = B // NCORES          # samples per core
HC = C // 2
TT = 512                  # conv tile width (one PSUM bank of f32)
NT = T // TT              # 16 tiles per sample
OUT_GROUP = 4             # tiles per output DMA (512 KiB of bf16)
E0 = 8                    # sample-0 tiles run in static-weight B mode

# sample-0 chunks: fine-grained, boundaries at tile*512+4 so each conv
# tile's 4-col halo stays within one chunk; alternate sync/gpsimd rings.
S0_BOUNDS = [0, 516, 1540, 3076, 5124, 7172, 8192]
S0_CONV = ["A", "V", "A", "V", "V", "A"]
# steady samples: coarse chunks, ACT+DVE converts (fused gap accum)
SB_BOUNDS = [0, 2048, 4096, 6144, 8192]
SB_CONV = ["A", "V", "V", "V"]
XF_W = 2048               # staging tile width (max chunk size)

FP32 = mybir.dt.float32
BF16 = mybir.dt.bfloat16

AF = mybir.ActivationFunctionType
ALU = mybir.AluOpType
AXL = mybir.AxisListType


def build_nc():
    nc = bacc.Bacc("TRN2", target_bir_lowering=False, debug=False)

    x_d = nc.dram_tensor("x", [BL, C, T], FP32, kind="ExternalInput").ap()
    il_d = nc.dram_tensor("invlen", [1, BL], FP32, kind="ExternalInput").ap()
    w1t_d = nc.dram_tensor("w1t", [C, 2 * C], BF16, kind="ExternalInput").ap()
    b1_d = nc.dram_tensor("b1", [1, 2 * C], FP32, kind="ExternalInput").ap()
    wt_d = nc.dram_tensor("wt", [C, K * C], BF16, kind="ExternalInput").ap()
    ones_d = nc.dram_tensor("ones", [1, 64], BF16, kind="ExternalInput").ap()
    out_d = nc.dram_tensor("out", [BL, C, T], BF16, kind="ExternalOutput").ap()

    with ExitStack() as ctx:
        tc = ctx.enter_context(tile.TileContext(nc))

        const = ctx.enter_context(tc.tile_pool(name="const", bufs=1))
        xf = ctx.enter_context(tc.tile_pool(name="xf", bufs=8))
        xb = ctx.enter_context(tc.tile_pool(name="xb", bufs=2))
        outp = ctx.enter_context(tc.tile_pool(name="outp", bufs=6))
        tmpp = ctx.enter_context(tc.tile_pool(name="tmpp", bufs=3))
        earlyp = ctx.enter_context(tc.tile_pool(name="early", bufs=E0))
        wscp = ctx.enter_context(tc.tile_pool(name="wscp", bufs=2))
        small = ctx.enter_context(tc.tile_pool(name="small", bufs=3))
        pc = ctx.enter_context(tc.tile_pool(name="pc", bufs=3, space="PSUM"))
        ps = ctx.enter_context(tc.tile_pool(name="ps", bufs=1, space="PSUM"))
        ph = ctx.enter_context(tc.tile_pool(name="ph", bufs=1, space="PSUM"))

        # constants ride the sync ring ahead of the x stream (small: ~230KB)
        # constants ride the scalar ring so the sync ring's FIFO starts
        # with sample 0's first x chunk (critical path to the first matmul)
        wt_sb = const.tile([C, K * C], BF16)
        nc.scalar.dma_start(wt_sb[:], wt_d[:])
        w1t_sb = const.tile([C, 2 * C], BF16)
        nc.scalar.dma_start(w1t_sb[:], w1t_d[:])
        b1_sb = const.tile([1, 2 * C], FP32)
        nc.scalar.dma_start(b1_sb[:], b1_d[:])
        il_sb = const.tile([1, BL], FP32)
        nc.scalar.dma_start(il_sb[:], il_d[:])
        ones_sb = const.tile([1, 64], BF16)
        nc.scalar.dma_start(ones_sb[:], ones_d[:])

        def emit_load(b):
            """DMA x[b] in chunks; convert to bf16 (+zero halo); row-sums."""
            first = b == 0
            bounds = S0_BOUNDS if first else SB_BOUNDS
            engs = S0_CONV if first else SB_CONV
            nch = len(bounds) - 1
            x_b = xb.tile([C, T + 2 * PAD], BF16)
            gap_parts = small.tile([C, 8], FP32, tag="gapp")
            nc.gpsimd.memset(x_b[:, 0:PAD], 0.0)
            nc.gpsimd.memset(x_b[:, T + PAD : T + 2 * PAD], 0.0)
            for c in range(nch):
                lo, hi = bounds[c], bounds[c + 1]
                w = hi - lo
                x_f = xf.tile([C, XF_W], FP32, tag="xfc")
                nc.sync.dma_start(x_f[:, 0:w], x_d[b, :, lo:hi])
                dst = x_b[:, PAD + lo : PAD + hi]
                src = x_f[:, 0:w]
                if engs[c] == "A":
                    nc.scalar.activation(
                        dst, src, AF.Copy, accum_out=gap_parts[:, c : c + 1]
                    )
                else:
                    nc.vector.tensor_scalar(
                        dst, src, 1.0, None,
                        op0=ALU.mult, op1=ALU.add,
                        accum_out=gap_parts[:, c : c + 1],
                    )
            return x_b, gap_parts, nch

        def emit_h(b, gap_parts, nch):
            """h_sb = sigmoid(gap @ w1.T * invlen + b1) as [1,2C] bf16."""
            gap_r = small.tile([C, 1], FP32, tag="gapr")
            nc.vector.tensor_reduce(
                gap_r[:], gap_parts[:, 0:nch], axis=AXL.X, op=ALU.add
            )
            gap_bf = small.tile([C, 1], BF16, tag="gapbf")
            nc.vector.tensor_copy(gap_bf[:], gap_r[:])
            hb = ph.tile([C, 2 * C + 2], FP32)
            h_ps = hb[0:1, 0 : 2 * C]
            nc.tensor.matmul(
                h_ps, lhsT=gap_bf[:], rhs=w1t_sb[:],
                start=True, stop=True, skip_group_check=True,
            )
            h_pre = small.tile([1, 2 * C], FP32, tag="hpre")
            nc.vector.scalar_tensor_tensor(
                h_pre[:], h_ps, il_sb[0:1, b : b + 1], b1_sb[:],
                op0=ALU.mult, op1=ALU.add,
            )
            h_sb = small.tile([1, 2 * C], BF16, tag="h")
            nc.scalar.activation(h_sb[:], h_pre[:], AF.Sigmoid)
            return h_sb, hb

        def emit_heho(h_sb, hb):
            """h_even/h_odd as per-partition [C,1] f32 columns (sample 0)."""
            h3 = h_sb[:].rearrange("p (a two) -> p two a", two=2)  # (1, 2, 128)
            heho_ps = hb[:, 2 * C : 2 * C + 2]
            nc.tensor.matmul(
                heho_ps[:, 0:1], lhsT=h3[:, 0, :], rhs=ones_sb[0:1, 0:1],
                start=True, stop=True, skip_group_check=True,
            )
            nc.tensor.matmul(
                heho_ps[:, 1:2], lhsT=h3[:, 1, :], rhs=ones_sb[0:1, 0:1],
                start=True, stop=True, skip_group_check=True,
            )
            heho_sb = small.tile([C, 2], FP32, tag="heho")
            nc.scalar.copy(heho_sb[:], heho_ps[:])
            return heho_sb

        def emit_wsc(h_sb):
            """S_b[ci,co] = h[2co + (ci>=64)] broadcast; wsc = W * S_b."""
            h3 = h_sb[:].rearrange("p (a two) -> p two a", two=2)  # (1, 2, 128)
            s_ps = ps.tile([C, C], FP32)
            nc.tensor.matmul(
                s_ps[0:64, :], lhsT=ones_sb[:], rhs=h3[:, 0, :],
                start=True, stop=True, skip_group_check=True,
            )
            nc.tensor.matmul(
                s_ps[64:128, :], lhsT=ones_sb[:], rhs=h3[:, 1, :],
                start=True, stop=True, skip_group_check=True,
            )
            wsc = wscp.tile([C, K * C], BF16)
            for k in range(K):
                nc.vector.tensor_mul(
                    wsc[:, k * C : (k + 1) * C],
                    wt_sb[:, k * C : (k + 1) * C],
                    s_ps[:],
                )
            return wsc

        def emit_conv_tile_b(x_b, t, buf):
            """B mode: 5 shifted 64-deep pairs, concurrent via row tiling."""
            base = t * TT
            for k in range(K):
                nc.tensor.matmul(
                    buf[:, 0:TT],
                    lhsT=wt_sb[0:HC, k * C : (k + 1) * C],
                    rhs=x_b[0:HC, base + k : base + k + TT],
                    start=(k == 0), stop=(k == K - 1),
                    skip_group_check=True,
                )
                nc.tensor.matmul(
                    buf[:, TT : 2 * TT],
                    lhsT=wt_sb[HC:C, k * C : (k + 1) * C],
                    rhs=x_b[HC:C, base + k : base + k + TT],
                    start=(k == 0), stop=(k == K - 1),
                    skip_group_check=True,
                )

        def emit_conv_pair_a(x_b, wsc, t0, buf):
            """A mode: tiles t0,t0+1 as 128-deep matmuls into one psum buf."""
            for half in range(2):
                base = (t0 + half) * TT
                for k in range(K):
                    nc.tensor.matmul(
                        buf[:, half * TT : (half + 1) * TT],
                        lhsT=wsc[:, k * C : (k + 1) * C],
                        rhs=x_b[:, base + k : base + k + TT],
                        start=(k == 0), stop=(k == K - 1),
                        skip_group_check=True,
                    )

        def drain_early(buf):
            """B mode, h not ready yet: park both psum halves in sbuf bf16."""
            eb = earlyp.tile([C, 2 * TT], BF16, tag="e")
            nc.scalar.copy(eb[:, TT : 2 * TT], buf[:, TT : 2 * TT])
            nc.vector.tensor_copy(eb[:, 0:TT], buf[:, 0:TT])
            return eb

        def combine_early(eb, heho_sb, o_sb, off):
            """out = h_even*lo + h_odd*hi from the parked bf16 halves."""
            u = tmpp.tile([C, TT], BF16, tag="u")
            nc.scalar.mul(u[:], eb[:, TT : 2 * TT], heho_sb[:, 1:2])
            nc.vector.scalar_tensor_tensor(
                o_sb[:, off : off + TT], eb[:, 0:TT], heho_sb[:, 0:1], u[:],
                op0=ALU.mult, op1=ALU.add,
            )

        def emit_out_dma(b, g, o_sb, split_tail):
            lo = g * OUT_GROUP * TT
            if split_tail:
                # drain the final group in 2-tile pieces so the kernel
                # tail isn't gated on one large DMA
                half = OUT_GROUP * TT // 2
                nc.gpsimd.dma_start(out_d[b, :, lo : lo + half], o_sb[:, 0:half])
                nc.gpsimd.dma_start(
                    out_d[b, :, lo + half : lo + 2 * half], o_sb[:, half:]
                )
            else:
                nc.gpsimd.dma_start(
                    out_d[b, :, lo : lo + OUT_GROUP * TT], o_sb[:]
                )

        # ---- sample 0 ----
        x_b0, gp0, nch0 = emit_load(0)
        early_bufs = []
        for t in range(E0):
            buf = pc.tile([C, 2 * TT], FP32)
            emit_conv_tile_b(x_b0, t, buf)
            early_bufs.append(drain_early(buf))
        h0, hb0 = emit_h(0, gp0, nch0)
        heho0 = emit_heho(h0, hb0)
        wsc0 = emit_wsc(h0)
        x_b1, gp1, nch1 = emit_load(1)

        o_sbs0 = [
            outp.tile([C, OUT_GROUP * TT], BF16, name="osb0", tag="osb")
            for _ in range(NT // OUT_GROUP)
        ]
        # steady A-mode conv for the rest of sample 0
        for t0 in range(E0, NT, 2):
            buf = pc.tile([C, 2 * TT], FP32)
            emit_conv_pair_a(x_b0, wsc0, t0, buf)
            g = t0 // OUT_GROUP
            nc.scalar.copy(
                o_sbs0[g][:, (t0 % OUT_GROUP) * TT : (t0 % OUT_GROUP + 2) * TT],
                buf[:],
            )
        for g in range(E0 // OUT_GROUP, NT // OUT_GROUP):
            emit_out_dma(0, g, o_sbs0[g], split_tail=False)

        # ---- samples 1..BL-1 (A mode); sample-0 combines spread across ----
        h1, _ = emit_h(1, gp1, nch1)
        wsc_cur = emit_wsc(h1)
        x_b_cur = x_b1
        for b in range(1, BL):
            nxt = None
            if b + 1 < BL:
                x_b_n, gp_n, nch_n = emit_load(b + 1)
                h_n, _ = emit_h(b + 1, gp_n, nch_n)
                nxt = (x_b_n, emit_wsc(h_n))
            last = b == BL - 1
            for g in range(NT // OUT_GROUP):
                o_sb = outp.tile([C, OUT_GROUP * TT], BF16, name="osb", tag="osb")
                for j2 in range(0, OUT_GROUP, 2):
                    buf = pc.tile([C, 2 * TT], FP32)
                    emit_conv_pair_a(x_b_cur, wsc_cur, g * OUT_GROUP + j2, buf)
                    nc.scalar.copy(o_sb[:, j2 * TT : (j2 + 2) * TT], buf[:])
                emit_out_dma(
                    b, g, o_sb, split_tail=(last and g == NT // OUT_GROUP - 1)
                )
            # 4 of sample 0's early combines per later sample (low priority)
            clo, chi = (b - 1) * 4, min(b * 4, E0)
            for t in range(clo, chi):
                combine_early(
                    early_bufs[t], heho0,
                    o_sbs0[t // OUT_GROUP], (t % OUT_GROUP) * TT,
                )
            if clo < chi and chi % OUT_GROUP == 0:
                emit_out_dma(0, chi // OUT_GROUP - 1, o_sbs0[chi // OUT_GROUP - 1],
                             split_tail=False)
            if nxt is not None:
                x_b_cur, wsc_cur = nxt

    nc.compile()
    return nc


_NC_CACHE = None


def _get_nc():
    global _NC_CACHE
    if _NC_CACHE is None:
        _NC_CACHE = build_nc()
    return _NC_CACHE


def make_in_maps(x, input_lengths, w1, b1, w2):
    import ml_dtypes

    x = np.ascontiguousarray(np.asarray(x, dtype=np.float32))
    lens = np.asarray(input_lengths).astype(np.float64)
    invlen = (1.0 / lens).astype(np.float32)
    w1t = np.ascontiguousarray(
        np.asarray(w1, dtype=np.float32).T.astype(ml_dtypes.bfloat16)
    )  # (C, 2C) bf16
    b1r = np.asarray(b1, dtype=np.float32).reshape(1, 2 * C)
    # wt[ci, k*C + co] = W[co, ci, k],  W = w2.reshape(C, C, K)
    wt = np.ascontiguousarray(
        np.asarray(w2, dtype=np.float32)
        .reshape(C, C, K)
        .transpose(1, 2, 0)
        .reshape(C, K * C)
        .astype(ml_dtypes.bfloat16)
    )
    ones = np.ones((1, 64), dtype=ml_dtypes.bfloat16)

    in_maps = []
    for i in range(NCORES):
        sl = slice(i * BL, (i + 1) * BL)
        in_maps.append(
            {
                "x": np.ascontiguousarray(x[sl]),
                "invlen": np.ascontiguousarray(invlen[sl].reshape(1, BL)),
                "w1t": w1t,
                "b1": b1r,
                "wt": wt,
                "ones": ones,
            }
        )
    return in_maps


def kernel(x, input_lengths, w1, b1, w2, _trace=False):
    nc = _get_nc()
    in_maps = make_in_maps(x, input_lengths, w1, b1, w2)
    res = run_bass_kernel_spmd(nc, in_maps, core_ids=list(range(NCORES)), trace=_trace)
    out = np.concatenate(
        [res.results[i]["out"].astype(np.float32) for i in range(NCORES)], axis=0
    )
    if _trace:
        kernel.last_exec_time_ns = res.exec_time_ns
        kernel.last_results = res
    return out
